# revision 36
# baseline (speedup 1.0000x reference)
"""Differentiable Gaussian renderer as a Trainium2 Bass kernel.

Strategy (self-contained; shapes hardcoded from the problem spec):
  - The image is split into 512 pixel tiles of 8x16 = 128 pixels; each
    tile's pixels live on the 128 SBUF partitions.  Tiles are dealt to the
    8 cores by descending gaussian count (round-robin), which equalizes
    the per-rank maxima that size the shared SPMD layout (L drops ~17%
    vs. band sharding).
  - Host prep (numpy, float64): project gaussians, depth-sort, and build a
    per-(core,tile) culled gaussian list (precise point-to-rectangle
    mahalanobis culling).  Tiles are packed back-to-back as
    [sep][g0..gC-1] segments in one dense column axis (identical layout on
    all 8 cores; per-rank capacity = max over cores), so one NEFF runs
    SPMD.  A subset-sum pick of slots puts a slot boundary exactly at
    column 512 so the transmittance scans split between engines.
  - Device, per 512-col chunk:
      Q = [gm;gm]^T @ [fhi;flo]   (ONE fp16 matmul, contraction 12)
      alpha = Exp(Q) on ACT -> f16 (Q <= 0 by construction; opacity is
      pre-clamped to 0.99 on host so no device clamp is needed)
      om = 1 - alpha (DVE for chunk 0, ACT Copy(scale=-1,bias=1) for 1/2)
      T = tensor_tensor_scan(om, inj, init=0) with separator-reset via max
        chunk 0 on DVE; chunks 1+2 on GpSimd (chained via initial AP)
      per 128-col block: PE transpose of T -> PSUM, ACT drain -> SBUF, and
      one small matmul per block against a block-sparse color-DIFFERENCE
      matrix (Abel summation: color = sum_col T[px,col]*d[col,ch], with
      d_sep = c_1, d_i = c_{i+1}-c_i, d_last = -c_n) accumulates every
      slot's [128px, 3] color into one PSUM bank -- no explicit w needed.
  - Input DMAs: fm on sync queue (single DMA, 12 rows), inj alone on the
    scalar queue (feeds the scans), ident + cb on the gpsimd queue.
  - Host unscrambles the [128, 192] per-core outputs into [3, 256, 256].
"""

import math
import numpy as np

H = W = 256
FX = FY = 300.0
CX = CY = 128.0
NEAR, FAR = 0.01, 100.0
TR, TC = 8, 16          # pixel tile shape (rows x cols); TR*TC == 128
NTY, NTX = H // TR, W // TC
NTILES = NTY * NTX      # 512 tiles over the full image
NSLOTS = NTILES // 8    # 64 tiles per core
NCORES = 8
QCUT = 5.0              # keep (gaussian, tile) if max_tile Q + log(opacity) > -QCUT
F_PAD = -88.0           # Q constant for separator / padding columns -> exp ~ 0

_compile_cache: dict = {}


def _host_prep(positions, scales, rotations, colors, opacities, view_matrix):
    N = positions.shape[0]
    f32 = np.float32

    # ---- depth sort exactly as the fp32 reference does ----
    pts_h32 = np.concatenate(
        [positions.astype(f32), np.ones((N, 1), f32)], axis=1)
    pcam32 = pts_h32 @ view_matrix.astype(f32).T
    x32, y32, z32 = pcam32[:, 0], pcam32[:, 1], pcam32[:, 2]
    depths32 = -z32
    order = np.argsort(depths32, kind="stable")

    # visibility mask in fp32 (must match reference's boundary decisions)
    z_safe32 = (np.clip(np.abs(z32), 0.01, None) *
                np.sign(z32 + f32(1e-8))).astype(f32)
    u32 = (f32(FX) * x32 / -z_safe32 + f32(CX)).astype(f32)
    v32 = (f32(FY) * -y32 / -z_safe32 + f32(CY)).astype(f32)
    vis = ((depths32 > NEAR) & (depths32 < FAR)
           & (u32 > -100) & (u32 < W + 100)
           & (v32 > -100) & (v32 < H + 100))

    # ---- float64 versions of the per-gaussian quantities ----
    pos = positions.astype(np.float64)
    sc = scales.astype(np.float64)
    rot = rotations.astype(np.float64)
    vm = view_matrix.astype(np.float64)
    q = rot / np.linalg.norm(rot, axis=-1, keepdims=True)
    qw, qx, qy, qz = q[:, 0], q[:, 1], q[:, 2], q[:, 3]
    Rm = np.stack([
        1 - 2*qy*qy - 2*qz*qz, 2*qx*qy - 2*qw*qz, 2*qx*qz + 2*qw*qy,
        2*qx*qy + 2*qw*qz, 1 - 2*qx*qx - 2*qz*qz, 2*qy*qz - 2*qw*qx,
        2*qx*qz - 2*qw*qy, 2*qy*qz + 2*qw*qx, 1 - 2*qx*qx - 2*qy*qy,
    ], axis=-1).reshape(N, 3, 3)
    pts = np.concatenate([pos, np.ones((N, 1))], 1) @ vm.T
    X, Y, Z = pts[:, 0], pts[:, 1], pts[:, 2]
    Rcam = np.einsum('ij,njk->nik', vm[:3, :3], Rm)
    RS = Rcam * sc[:, None, :]
    cov3d = RS @ np.swapaxes(RS, -1, -2)
    z_safe = np.clip(np.abs(Z), 0.01, None) * np.sign(Z + 1e-8)
    z2 = z_safe * z_safe
    J = np.zeros((N, 2, 3))
    J[:, 0, 0] = FX / -z_safe
    J[:, 0, 2] = FX * X / z2
    J[:, 1, 1] = FY / z_safe
    J[:, 1, 2] = FY * Y / z2
    cov2d = np.einsum('nij,njk,nlk->nil', J, cov3d, J)
    u = FX * X / -z_safe + CX
    v = FY * -Y / -z_safe + CY

    # sort everything front-to-back
    u, v, vis = u[order], v[order], vis[order]
    cov2d = cov2d[order]
    opa = opacities.astype(np.float64)[order]
    cols = colors.astype(np.float64)[order]

    a = cov2d[:, 0, 0] + 1e-4
    b = cov2d[:, 0, 1]
    c = cov2d[:, 1, 1] + 1e-4
    det = a * c - b * b
    ia2 = -0.5 * c / det
    ib2 = b / det
    ic2 = -0.5 * a / det
    keepable = vis & (opa > 0)
    # opacity pre-clamped at 0.99: replaces the reference's per-pixel
    # min(alpha, 0.99) (alpha = exp(Q) <= opacity <= 0.99 everywhere)
    opa_eff = np.minimum(opa, 0.99)
    logo = np.where(keepable, np.log(np.maximum(opa_eff, 1e-300)), -1e9)

    # ---- precise per-(core,tile) culling ----
    def qmax_tile(y0, x0):
        inside = (u >= x0) & (u <= x0 + TC - 1) & (v >= y0) & (v <= y0 + TR - 1)
        best = np.full(N, -np.inf)
        for xe in (x0, x0 + TC - 1):
            dx = xe - u
            dy_cl = np.clip(-ib2 * dx / (2 * ic2), y0 - v, y0 + TR - 1 - v)
            best = np.maximum(best, ia2*dx*dx + ib2*dx*dy_cl + ic2*dy_cl*dy_cl)
        for ye in (y0, y0 + TR - 1):
            dy = ye - v
            dx_cl = np.clip(-ib2 * dy / (2 * ia2), x0 - u, x0 + TC - 1 - u)
            best = np.maximum(best, ia2*dx_cl*dx_cl + ib2*dx_cl*dy + ic2*dy*dy)
        return np.where(inside, 0.0, best)

    # cull per global tile (any core can render any tile)
    keep = np.zeros((NTILES, N), bool)
    for t in range(NTILES):
        y0 = (t // NTX) * TR
        x0 = (t % NTX) * TC
        keep[t] = keepable & (qmax_tile(y0, x0) + logo > -QCUT)

    counts = keep.sum(axis=1)                      # [512]
    # deal tiles to cores by descending count: rank r takes the r-th group
    # of 8; the per-rank capacity is then the largest of that group, which
    # minimizes sum(caps) over all balanced assignments
    deal = np.argsort(-counts, kind="stable")
    slot_order = deal.reshape(NSLOTS, NCORES).T    # [8, 64] global tile ids
    caps = counts[deal[::NCORES]].astype(np.int64)           # [64] rank max
    sizes = caps + 1                               # incl. separator column

    # ---- pack: subset-sum slot groups so boundaries land exactly on the
    # scan chunk edges (512, 1024): scans then start fresh (initial=0)
    # instead of chaining; pad the tail so L is a multiple of 128 ----
    total = int(sizes.sum())
    bounds = [b for b in (512, 1024, ((total - 1) // 128) * 128) if b < total]
    remaining = list(range(NSLOTS))
    out_ranks = []
    for target in bounds:
        need = target - sum(int(sizes[r]) for r in out_ranks)
        if need <= 0:
            break
        sub = _subset_exact([int(sizes[r]) for r in remaining], need)
        if sub is None:
            continue
        picked = [remaining[i] for i in sub]
        out_ranks += picked
        remaining = [r for r in remaining if r not in set(picked)]
    out_ranks += remaining
    # out-order offsets; pad before any boundary a slot would straddle
    offs = np.zeros(NSLOTS, np.int64)              # by out-index
    col0 = 0
    for oi, r in enumerate(out_ranks):
        for bnd in bounds:
            if col0 < bnd and col0 + int(sizes[r]) > bnd:
                col0 = bnd                         # pad up to the boundary
        offs[oi] = col0
        col0 += int(sizes[r])
    L = ((col0 + 127) // 128) * 128                # pad tail to block boundary
    caps_o = np.array([caps[r] for r in out_ranks], np.int64)

    # scan chunks: PSUM-bank pieces further split at the last boundary so
    # the final chunk is small (short tail); out-chunk = slots fully inside
    ends = offs + 1 + caps_o
    schunks = []
    c0 = 0
    for bnd in sorted(set(bounds + [L])):
        if c0 < bnd:
            while bnd - c0 > 512:
                schunks.append((c0, c0 + 512))
                c0 += 512
            schunks.append((c0, bnd))
            c0 = bnd
    if c0 < L:
        schunks.append((c0, L))
    out_chunks = []
    prev = 0
    for (_s0, s1) in schunks:
        n = int(np.searchsorted(ends, s1, side="right"))
        out_chunks.append((prev, n))
        prev = n

    # ---- color-matmul blocks: per 128-col block, the (out-consecutive)
    # slots whose columns (incl. separator) intersect it ----
    nblocks = L // 128
    blocks = []          # (b, j0, j1, cb_off)  j in out-index
    cb_off = 0
    for bb in range(nblocks):
        lo, hi = bb * 128, bb * 128 + 128
        js = [j for j in range(NSLOTS)
              if offs[j] < hi and int(ends[j]) > lo]
        if not js:
            continue
        j0, j1 = min(js), max(js)
        assert js == list(range(j0, j1 + 1))
        blocks.append((bb, j0, j1, cb_off))
        cb_off += 3 * (j1 - j0 + 1)
    CB = max(cb_off, 1)

    # ---- packed per-core arrays ----
    fmat = np.zeros((NCORES, 6, L), f32)
    fmat[:, 5, :] = F_PAD
    colblk = np.zeros((NCORES, 128, CB), f32)

    for core in range(NCORES):
        for oi, r in enumerate(out_ranks):
            ti = int(slot_order[core, r])          # global tile id
            n = int(counts[ti])
            if n == 0:
                continue
            y0 = (ti // NTX) * TR
            x0 = (ti % NTX) * TC
            x0c = x0 + (TC - 1) / 2.0
            y0c = y0 + (TR - 1) / 2.0
            g = np.where(keep[ti])[0]              # sorted (front-to-back)
            up = u[g] - x0c
            vp = v[g] - y0c
            s = int(offs[oi]) + 1
            fmat[core, 0, s:s+n] = ia2[g]
            fmat[core, 1, s:s+n] = ib2[g]
            fmat[core, 2, s:s+n] = ic2[g]
            fmat[core, 3, s:s+n] = -2*ia2[g]*up - ib2[g]*vp
            fmat[core, 4, s:s+n] = -2*ic2[g]*vp - ib2[g]*up
            fmat[core, 5, s:s+n] = (ia2[g]*up*up + ib2[g]*up*vp
                                    + ic2[g]*vp*vp + logo[g])
            # color-difference rows (Abel summation): column s-1 (separator)
            # takes c_1; column s+i-1 takes c_{i+1}-c_i; column s+n-1 takes
            # -c_n; padding columns stay 0.
            cg = cols[g]                           # [n, 3]
            dcol = np.zeros((n + 1, 3))
            dcol[0] = cg[0]
            if n > 1:
                dcol[1:n] = cg[1:] - cg[:-1]
            dcol[n] = -cg[-1]
            # scatter dcol (columns s-1 .. s+n-1) into the block-sparse cb
            for bb, j0, j1, cbo in blocks:
                lo, hi = bb * 128, bb * 128 + 128
                a0 = max(s - 1, lo)
                a1 = min(s + n, hi)
                if a0 >= a1 or not (j0 <= oi <= j1):
                    continue
                rows = np.arange(a0 - lo, a1 - lo)
                src = dcol[a0 - (s - 1): a1 - (s - 1)]
                colblk[core, rows, cbo + 3 * (oi - j0) + 0] = src[:, 0]
                colblk[core, rows, cbo + 3 * (oi - j0) + 1] = src[:, 1]
                colblk[core, rows, cbo + 3 * (oi - j0) + 2] = src[:, 2]

    # pixel polynomial matrix, shared by every tile and core
    dr, dc = np.divmod(np.arange(128), TC)
    gx = (dc - (TC - 1) / 2.0).astype(f32)
    gy = (dr - (TR - 1) / 2.0).astype(f32)
    gm = np.stack([gx*gx, gx*gy, gy*gy, gx, gy, np.ones(128, f32)]).astype(f32)

    # fp16 split of F: F = hi + lo recovers ~21 mantissa bits; gm is exact
    # in fp16.  Both halves stacked so one contraction-12 matmul computes
    # gm^T @ fhi + gm^T @ flo per chunk.
    assert np.abs(fmat).max() < 16000.0, "F coefficients too large for fp16 split"
    inj = np.zeros(L, np.float16)
    inj[offs] = 1.0
    inj_rep = np.broadcast_to(inj, (128, L)).copy()

    gm2 = np.concatenate([gm, gm], axis=0).astype(np.float16)   # [12, 128]

    in_maps = []
    ident = np.eye(128, dtype=np.float16)
    for core in range(NCORES):
        fhi = fmat[core].astype(np.float16)
        flo = (fmat[core].astype(np.float64)
               - fhi.astype(np.float64)).astype(np.float16)
        fhilo = np.concatenate([fhi, flo], axis=0)              # [12, L]
        fmat_all = np.concatenate([gm2, fhilo], axis=1)         # [12, 128+L]
        in_maps.append({
            "fmat": np.ascontiguousarray(fmat_all),
            "colblk": np.ascontiguousarray(colblk[core].astype(np.float16)),
            "ident": ident,
            "inj": inj_rep,
        })
    meta = dict(L=L, blocks=tuple(blocks), CB=CB, out_chunks=tuple(out_chunks),
                schunks=tuple(schunks))
    return in_maps, meta, out_ranks, slot_order


def _subset_exact(sizes, target):
    """Pick slot ranks whose sizes sum to exactly `target` (DP subset-sum).
    Returns the picked ranks (ascending) or None."""
    n = len(sizes)
    reach = np.zeros((n + 1, target + 1), bool)
    reach[0, 0] = True
    for i in range(n):
        s = int(sizes[i])
        reach[i + 1] = reach[i].copy()
        if s <= target:
            reach[i + 1, s:] |= reach[i, :-s]
    if not reach[n, target]:
        return None
    picked = []
    t = target
    for i in range(n, 0, -1):
        s = int(sizes[i - 1])
        if reach[i - 1, t]:
            continue                               # item i-1 not needed
        picked.append(i - 1)
        t -= s
    assert t == 0
    return sorted(picked)


def _build_program(meta):
    import concourse.bacc as bacc
    import concourse.mybir as mybir
    from concourse.tile import TileContext
    from concourse.mybir import AluOpType

    L = meta["L"]
    blocks = meta["blocks"]
    CB = meta["CB"]
    out_chunks = meta["out_chunks"]

    f32 = mybir.dt.float32
    f16 = mybir.dt.float16
    fm_cols = 128 + L
    nc = bacc.Bacc("TRN2", target_bir_lowering=False)
    f_d = nc.dram_tensor("fmat", [12, fm_cols], f16, kind="ExternalInput")
    cb_d = nc.dram_tensor("colblk", [128, CB], f16, kind="ExternalInput")
    id_d = nc.dram_tensor("ident", [128, 128], f16, kind="ExternalInput")
    inj_d = nc.dram_tensor("inj", [128, L], f16, kind="ExternalInput")
    out_d = nc.dram_tensor("out", [128, 3 * NSLOTS], f32, kind="ExternalOutput")

    # compute chunks for Q/exp/om (PSUM-bank sized)
    qchunks = []
    c0 = 0
    while c0 < L:
        qchunks.append((c0, min(c0 + 512, L)))
        c0 += 512
    # scan chunks (from host packing), all on DVE (GpSimd lacks the scan
    # opcode on TRN2).  No slot straddles a chunk boundary, so every chunk
    # starts fresh with initial=0 -- no chaining.  Execution order is
    # pinned by the om chunks being serial on ACT.
    schunks = meta["schunks"]

    # a block's transpose is ready after the scan chunk covering its end
    blocks_by_schunk = [[] for _ in schunks]
    for blk in blocks:
        bend = (blk[0] + 1) * 128
        for si, (_c0, c1_) in enumerate(schunks):
            if bend <= c1_:
                blocks_by_schunk[si].append(blk)
                break

    with TileContext(nc) as tc:
        with (
            tc.tile_pool(name="const", bufs=1) as cpool,
            tc.tile_pool(name="wts", bufs=2) as wpool,
            tc.tile_pool(name="psum", bufs=3, space="PSUM") as ppool,
            tc.tile_pool(name="trps", bufs=2, space="PSUM") as tpool,
            tc.tile_pool(name="colps", bufs=1, space="PSUM") as opool,
        ):
            fm = cpool.tile([12, fm_cols], f16)
            inj = cpool.tile([128, L], f16)
            cb = cpool.tile([128, CB], f16)
            ident = cpool.tile([128, 128], f16)
            # Everything HWDGE rides the sync queue in consumer order: fm's
            # 12 descriptors enter the DMA rings first (the first matmul
            # gates on them), then inj in two pieces so scan0's gate fires
            # as soon as its half lands, then cb.  Any DMA on the scalar
            # queue floods the shared rings before fm (its gen starts
            # immediately at program entry), so the scalar queue carries no
            # input DMAs at all.
            s_fm = 128 + min(512, L)
            s_inj = min(512, L)
            nc.sync.dma_start(fm[:, :s_fm], f_d[:, :s_fm])
            nc.sync.dma_start(fm[:, s_fm:], f_d[:, s_fm:])
            nc.sync.dma_start(inj[:, :s_inj], inj_d[:, :s_inj])
            if s_inj < L:
                nc.sync.dma_start(inj[:, s_inj:], inj_d[:, s_inj:])
            nc.gpsimd.dma_start(ident[:, :], id_d[:, :])
            nc.sync.dma_start(cb[:, :], cb_d[:, :])

            gm2 = fm[:, 0:128]
            alphat = cpool.tile([128, L], f16)
            omap = cpool.tile([128, L], f16)
            Tt = cpool.tile([128, L], f16)
            colb = cpool.tile([128, 3 * NSLOTS], f32)
            colps = opool.tile([128, 3 * NSLOTS], f32)

            nc.vector.memset(colps[:, :], 0.0)

            # Q matmul + exp per PSUM-bank chunk
            for qi, (c0, c1) in enumerate(qchunks):
                n = c1 - c0
                ps = ppool.tile([128, 512], f32, tag="ps", name="ps")
                nc.tensor.matmul(ps[:, :n], gm2, fm[:, 128 + c0:128 + c1],
                                 start=True, stop=True)
                nc.scalar.activation(alphat[:, c0:c1], ps[:, :n],
                                     mybir.ActivationFunctionType.Exp)

            # om = 1 - alpha per scan chunk (alpha <= 0.99 by host pre-clamp
            # of opacity; Q <= 0 so exp <= 1).  Chunk 0 on DVE (feeds its
            # scan immediately), later chunks on ACT so DVE runs scans
            # back-to-back; the serial ACT order also pins scan order.
            for si, (c0, c1) in enumerate(schunks):
                if si == 0:
                    nc.vector.tensor_scalar(omap[:, c0:c1], alphat[:, c0:c1],
                                            -1.0, 1.0, AluOpType.mult,
                                            AluOpType.add)
                else:
                    nc.scalar.activation(omap[:, c0:c1], alphat[:, c0:c1],
                                         mybir.ActivationFunctionType.Copy,
                                         bias=1.0, scale=-1.0)

            for si, (c0, c1) in enumerate(schunks):
                nc.vector.tensor_tensor_scan(Tt[:, c0:c1], omap[:, c0:c1],
                                             inj[:, c0:c1], 0.0,
                                             AluOpType.mult, AluOpType.max)

                # blocks finalized by this scan: PE transpose -> PSUM,
                # drain -> SBUF, small matmuls accumulate colors, then
                # flush the finalized slots (clamp + store)
                blks = blocks_by_schunk[si]
                last_chunk = si == len(schunks) - 1
                if blks:
                    nb = len(blks)
                    trp = tpool.tile([128, 512], f16, tag="trp", name="trp")
                    wT = wpool.tile([128, 512], f16, tag="wT", name="wT")
                    for t, (bb, j0, j1, cbo) in enumerate(blks):
                        lo = bb * 128
                        nc.tensor.transpose(trp[:, 128 * t:128 * t + 128],
                                            Tt[:, lo:lo + 128], ident[:, :])
                    span = 128 * nb
                    if last_chunk:
                        # DVE is idle once the scans finish; draining here
                        # skips the ACT queue (still busy with drains 0/1)
                        nc.vector.tensor_copy(wT[:, :span], trp[:, :span])
                    else:
                        nc.scalar.copy(wT[:, :span], trp[:, :span])
                    for t, (bb, j0, j1, cbo) in enumerate(blks):
                        k3 = 3 * (j1 - j0 + 1)
                        nc.tensor.matmul(colps[:, 3 * j0: 3 * j0 + k3],
                                         wT[:, 128 * t:128 * t + 128],
                                         cb[:, cbo: cbo + k3],
                                         start=False, stop=False,
                                         skip_group_check=True)
                # slots fully covered by this chunk are final once its
                # color matmuls are issued; the last chunk's flush rides the
                # scalar queue so its descriptor gen runs in parallel with
                # the sync queue's earlier output DMAs
                oA, oB = out_chunks[si]
                if oB > oA:
                    nc.vector.tensor_scalar(colb[:, 3 * oA:3 * oB],
                                            colps[:, 3 * oA:3 * oB],
                                            0.0, 1.0,
                                            AluOpType.max, AluOpType.min)
                    dq = nc.scalar if last_chunk else nc.sync
                    dq.dma_start(out_d[:, 3 * oA:3 * oB],
                                 colb[:, 3 * oA:3 * oB])
    nc.finalize()
    return nc


def _assemble(results, out_ranks, slot_order):
    out = np.zeros((3, H, W), np.float32)
    dr, dc = np.divmod(np.arange(128), TC)
    for core in range(NCORES):
        o = np.clip(results[core]["out"], 0.0, 1.0)   # [128, 192]
        for oi, r in enumerate(out_ranks):
            ti = int(slot_order[core, r])          # global tile id
            y0 = (ti // NTX) * TR
            x0 = (ti % NTX) * TC
            for ch in range(3):
                out[ch, y0 + dr, x0 + dc] = o[:, 3 * oi + ch]
    return out


def _run(inputs, trace=False, trace_cores=None):
    in_maps, meta, out_ranks, slot_order = _host_prep(
        inputs["positions"], inputs["scales"], inputs["rotations"],
        inputs["colors"], inputs["opacities"], inputs["view_matrix"])

    key = (meta["L"], meta["blocks"], meta["out_chunks"], meta["schunks"])
    if key not in _compile_cache:
        _compile_cache[key] = _build_program(meta)
    nc = _compile_cache[key]

    from concourse.bass_utils import run_bass_kernel_spmd
    kw = {}
    if trace:
        kw = dict(trace=True,
                  trace_cores=trace_cores or list(range(NCORES)))
    res = run_bass_kernel_spmd(nc, in_maps, core_ids=list(range(NCORES)), **kw)
    return _assemble(res.results, out_ranks, slot_order), res


def kernel(**inputs) -> np.ndarray:
    out, _ = _run(inputs, trace=False)
    return out


# revision 41
# speedup vs baseline: 1.0248x; 1.0248x over previous
"""Differentiable Gaussian renderer as a Trainium2 Bass kernel.

Strategy (self-contained; shapes hardcoded from the problem spec):
  - The image is split into 512 pixel tiles of 8x16 = 128 pixels; each
    tile's pixels live on the 128 SBUF partitions.  Tiles are dealt to the
    8 cores by descending gaussian count (round-robin), which equalizes
    the per-rank maxima that size the shared SPMD layout (L drops ~17%
    vs. band sharding).
  - Host prep (numpy, float64): project gaussians, depth-sort, and build a
    per-(core,tile) culled gaussian list (precise point-to-rectangle
    mahalanobis culling).  Tiles are packed back-to-back as
    [sep][g0..gC-1] segments in one dense column axis (identical layout on
    all 8 cores; per-rank capacity = max over cores), so one NEFF runs
    SPMD.  A subset-sum pick of slots puts a slot boundary exactly at
    column 512 so the transmittance scans split between engines.
  - Device, per 512-col chunk:
      Q = [gm;gm]^T @ [fhi;flo]   (ONE fp16 matmul, contraction 12)
      alpha = Exp(Q) on ACT -> f16 (Q <= 0 by construction; opacity is
      pre-clamped to 0.99 on host so no device clamp is needed)
      om = 1 - alpha (DVE for chunk 0, ACT Copy(scale=-1,bias=1) for 1/2)
      T = tensor_tensor_scan(om, inj, init=0) with separator-reset via max
        chunk 0 on DVE; chunks 1+2 on GpSimd (chained via initial AP)
      per 128-col block: PE transpose of T -> PSUM, ACT drain -> SBUF, and
      one small matmul per block against a block-sparse color-DIFFERENCE
      matrix (Abel summation: color = sum_col T[px,col]*d[col,ch], with
      d_sep = c_1, d_i = c_{i+1}-c_i, d_last = -c_n) accumulates every
      slot's [128px, 3] color into one PSUM bank -- no explicit w needed.
  - Input DMAs: fm on sync queue (single DMA, 12 rows), inj alone on the
    scalar queue (feeds the scans), ident + cb on the gpsimd queue.
  - Host unscrambles the [128, 192] per-core outputs into [3, 256, 256].
"""

import math
import numpy as np

H = W = 256
FX = FY = 300.0
CX = CY = 128.0
NEAR, FAR = 0.01, 100.0
TR, TC = 8, 16          # pixel tile shape (rows x cols); TR*TC == 128
NTY, NTX = H // TR, W // TC
NTILES = NTY * NTX      # 512 tiles over the full image
NSLOTS = NTILES // 8    # 64 tiles per core
NCORES = 8
QCUT = 5.0              # keep (gaussian, tile) if max_tile Q + log(opacity) > -QCUT
F_PAD = -88.0           # Q constant for separator / padding columns -> exp ~ 0

_compile_cache: dict = {}


def _host_prep(positions, scales, rotations, colors, opacities, view_matrix):
    N = positions.shape[0]
    f32 = np.float32

    # ---- depth sort exactly as the fp32 reference does ----
    pts_h32 = np.concatenate(
        [positions.astype(f32), np.ones((N, 1), f32)], axis=1)
    pcam32 = pts_h32 @ view_matrix.astype(f32).T
    x32, y32, z32 = pcam32[:, 0], pcam32[:, 1], pcam32[:, 2]
    depths32 = -z32
    order = np.argsort(depths32, kind="stable")

    # visibility mask in fp32 (must match reference's boundary decisions)
    z_safe32 = (np.clip(np.abs(z32), 0.01, None) *
                np.sign(z32 + f32(1e-8))).astype(f32)
    u32 = (f32(FX) * x32 / -z_safe32 + f32(CX)).astype(f32)
    v32 = (f32(FY) * -y32 / -z_safe32 + f32(CY)).astype(f32)
    vis = ((depths32 > NEAR) & (depths32 < FAR)
           & (u32 > -100) & (u32 < W + 100)
           & (v32 > -100) & (v32 < H + 100))

    # ---- float64 versions of the per-gaussian quantities ----
    pos = positions.astype(np.float64)
    sc = scales.astype(np.float64)
    rot = rotations.astype(np.float64)
    vm = view_matrix.astype(np.float64)
    q = rot / np.linalg.norm(rot, axis=-1, keepdims=True)
    qw, qx, qy, qz = q[:, 0], q[:, 1], q[:, 2], q[:, 3]
    Rm = np.stack([
        1 - 2*qy*qy - 2*qz*qz, 2*qx*qy - 2*qw*qz, 2*qx*qz + 2*qw*qy,
        2*qx*qy + 2*qw*qz, 1 - 2*qx*qx - 2*qz*qz, 2*qy*qz - 2*qw*qx,
        2*qx*qz - 2*qw*qy, 2*qy*qz + 2*qw*qx, 1 - 2*qx*qx - 2*qy*qy,
    ], axis=-1).reshape(N, 3, 3)
    pts = np.concatenate([pos, np.ones((N, 1))], 1) @ vm.T
    X, Y, Z = pts[:, 0], pts[:, 1], pts[:, 2]
    Rcam = np.einsum('ij,njk->nik', vm[:3, :3], Rm)
    RS = Rcam * sc[:, None, :]
    cov3d = RS @ np.swapaxes(RS, -1, -2)
    z_safe = np.clip(np.abs(Z), 0.01, None) * np.sign(Z + 1e-8)
    z2 = z_safe * z_safe
    J = np.zeros((N, 2, 3))
    J[:, 0, 0] = FX / -z_safe
    J[:, 0, 2] = FX * X / z2
    J[:, 1, 1] = FY / z_safe
    J[:, 1, 2] = FY * Y / z2
    cov2d = np.einsum('nij,njk,nlk->nil', J, cov3d, J)
    u = FX * X / -z_safe + CX
    v = FY * -Y / -z_safe + CY

    # sort everything front-to-back
    u, v, vis = u[order], v[order], vis[order]
    cov2d = cov2d[order]
    opa = opacities.astype(np.float64)[order]
    cols = colors.astype(np.float64)[order]

    a = cov2d[:, 0, 0] + 1e-4
    b = cov2d[:, 0, 1]
    c = cov2d[:, 1, 1] + 1e-4
    det = a * c - b * b
    ia2 = -0.5 * c / det
    ib2 = b / det
    ic2 = -0.5 * a / det
    keepable = vis & (opa > 0)
    # opacity pre-clamped at 0.99: replaces the reference's per-pixel
    # min(alpha, 0.99) (alpha = exp(Q) <= opacity <= 0.99 everywhere)
    opa_eff = np.minimum(opa, 0.99)
    logo = np.where(keepable, np.log(np.maximum(opa_eff, 1e-300)), -1e9)

    # ---- precise per-(core,tile) culling ----
    def qmax_tile(y0, x0):
        inside = (u >= x0) & (u <= x0 + TC - 1) & (v >= y0) & (v <= y0 + TR - 1)
        best = np.full(N, -np.inf)
        for xe in (x0, x0 + TC - 1):
            dx = xe - u
            dy_cl = np.clip(-ib2 * dx / (2 * ic2), y0 - v, y0 + TR - 1 - v)
            best = np.maximum(best, ia2*dx*dx + ib2*dx*dy_cl + ic2*dy_cl*dy_cl)
        for ye in (y0, y0 + TR - 1):
            dy = ye - v
            dx_cl = np.clip(-ib2 * dy / (2 * ia2), x0 - u, x0 + TC - 1 - u)
            best = np.maximum(best, ia2*dx_cl*dx_cl + ib2*dx_cl*dy + ic2*dy*dy)
        return np.where(inside, 0.0, best)

    # cull per global tile (any core can render any tile)
    keep = np.zeros((NTILES, N), bool)
    for t in range(NTILES):
        y0 = (t // NTX) * TR
        x0 = (t % NTX) * TC
        keep[t] = keepable & (qmax_tile(y0, x0) + logo > -QCUT)

    counts = keep.sum(axis=1)                      # [512]
    # deal tiles to cores by descending count: rank r takes the r-th group
    # of 8; the per-rank capacity is then the largest of that group, which
    # minimizes sum(caps) over all balanced assignments
    deal = np.argsort(-counts, kind="stable")
    slot_order = deal.reshape(NSLOTS, NCORES).T    # [8, 64] global tile ids
    caps = counts[deal[::NCORES]].astype(np.int64)           # [64] rank max
    sizes = caps + 1                               # incl. separator column

    # ---- pack: subset-sum slot groups so boundaries land exactly on the
    # scan chunk edges (512, 1024): scans then start fresh (initial=0)
    # instead of chaining; pad the tail so L is a multiple of 128 ----
    total = int(sizes.sum())
    bounds = [b for b in (512, 896, 1152) if b < total]
    remaining = list(range(NSLOTS))
    out_ranks = []
    for target in bounds:
        need = target - sum(int(sizes[r]) for r in out_ranks)
        if need <= 0:
            break
        sub = _subset_exact([int(sizes[r]) for r in remaining], need)
        if sub is None:
            continue
        picked = [remaining[i] for i in sub]
        out_ranks += picked
        remaining = [r for r in remaining if r not in set(picked)]
    out_ranks += remaining
    # out-order offsets; pad before any boundary a slot would straddle
    offs = np.zeros(NSLOTS, np.int64)              # by out-index
    col0 = 0
    for oi, r in enumerate(out_ranks):
        for bnd in bounds:
            if col0 < bnd and col0 + int(sizes[r]) > bnd:
                col0 = bnd                         # pad up to the boundary
        offs[oi] = col0
        col0 += int(sizes[r])
    L = ((col0 + 127) // 128) * 128                # pad tail to block boundary
    caps_o = np.array([caps[r] for r in out_ranks], np.int64)

    # scan chunks: PSUM-bank pieces further split at the last boundary so
    # the final chunk is small (short tail); out-chunk = slots fully inside
    ends = offs + 1 + caps_o
    schunks = []
    c0 = 0
    for bnd in sorted(set(bounds + [L])):
        if c0 < bnd:
            while bnd - c0 > 512:
                schunks.append((c0, c0 + 512))
                c0 += 512
            schunks.append((c0, bnd))
            c0 = bnd
    if c0 < L:
        schunks.append((c0, L))
    out_chunks = []
    prev = 0
    for (_s0, s1) in schunks:
        n = int(np.searchsorted(ends, s1, side="right"))
        out_chunks.append((prev, n))
        prev = n

    # ---- color-matmul blocks: per 128-col block, the (out-consecutive)
    # slots whose columns (incl. separator) intersect it ----
    nblocks = L // 128
    blocks = []          # (b, j0, j1, cb_off)  j in out-index
    cb_off = 0
    for bb in range(nblocks):
        lo, hi = bb * 128, bb * 128 + 128
        js = [j for j in range(NSLOTS)
              if offs[j] < hi and int(ends[j]) > lo]
        if not js:
            continue
        j0, j1 = min(js), max(js)
        assert js == list(range(j0, j1 + 1))
        blocks.append((bb, j0, j1, cb_off))
        cb_off += 3 * (j1 - j0 + 1)
    CB = max(cb_off, 1)

    # ---- packed per-core arrays ----
    fmat = np.zeros((NCORES, 6, L), f32)
    fmat[:, 5, :] = F_PAD
    colblk = np.zeros((NCORES, 128, CB), f32)

    for core in range(NCORES):
        for oi, r in enumerate(out_ranks):
            ti = int(slot_order[core, r])          # global tile id
            n = int(counts[ti])
            if n == 0:
                continue
            y0 = (ti // NTX) * TR
            x0 = (ti % NTX) * TC
            x0c = x0 + (TC - 1) / 2.0
            y0c = y0 + (TR - 1) / 2.0
            g = np.where(keep[ti])[0]              # sorted (front-to-back)
            up = u[g] - x0c
            vp = v[g] - y0c
            s = int(offs[oi]) + 1
            fmat[core, 0, s:s+n] = ia2[g]
            fmat[core, 1, s:s+n] = ib2[g]
            fmat[core, 2, s:s+n] = ic2[g]
            fmat[core, 3, s:s+n] = -2*ia2[g]*up - ib2[g]*vp
            fmat[core, 4, s:s+n] = -2*ic2[g]*vp - ib2[g]*up
            fmat[core, 5, s:s+n] = (ia2[g]*up*up + ib2[g]*up*vp
                                    + ic2[g]*vp*vp + logo[g])
            # color-difference rows (Abel summation): column s-1 (separator)
            # takes c_1; column s+i-1 takes c_{i+1}-c_i; column s+n-1 takes
            # -c_n; padding columns stay 0.
            cg = cols[g]                           # [n, 3]
            dcol = np.zeros((n + 1, 3))
            dcol[0] = cg[0]
            if n > 1:
                dcol[1:n] = cg[1:] - cg[:-1]
            dcol[n] = -cg[-1]
            # scatter dcol (columns s-1 .. s+n-1) into the block-sparse cb
            for bb, j0, j1, cbo in blocks:
                lo, hi = bb * 128, bb * 128 + 128
                a0 = max(s - 1, lo)
                a1 = min(s + n, hi)
                if a0 >= a1 or not (j0 <= oi <= j1):
                    continue
                rows = np.arange(a0 - lo, a1 - lo)
                src = dcol[a0 - (s - 1): a1 - (s - 1)]
                colblk[core, rows, cbo + 3 * (oi - j0) + 0] = src[:, 0]
                colblk[core, rows, cbo + 3 * (oi - j0) + 1] = src[:, 1]
                colblk[core, rows, cbo + 3 * (oi - j0) + 2] = src[:, 2]

    # pixel polynomial matrix, shared by every tile and core
    dr, dc = np.divmod(np.arange(128), TC)
    gx = (dc - (TC - 1) / 2.0).astype(f32)
    gy = (dr - (TR - 1) / 2.0).astype(f32)
    gm = np.stack([gx*gx, gx*gy, gy*gy, gx, gy, np.ones(128, f32)]).astype(f32)

    # fp16 split of F: F = hi + lo recovers ~21 mantissa bits; gm is exact
    # in fp16.  Both halves stacked so one contraction-12 matmul computes
    # gm^T @ fhi + gm^T @ flo per chunk.
    assert np.abs(fmat).max() < 16000.0, "F coefficients too large for fp16 split"
    inj = np.zeros(L, np.float16)
    inj[offs] = 1.0
    inj_rep = np.broadcast_to(inj, (128, L)).copy()

    gm2 = np.concatenate([gm, gm], axis=0).astype(np.float16)   # [12, 128]

    in_maps = []
    ident = np.eye(128, dtype=np.float16)
    for core in range(NCORES):
        fhi = fmat[core].astype(np.float16)
        flo = (fmat[core].astype(np.float64)
               - fhi.astype(np.float64)).astype(np.float16)
        fhilo = np.concatenate([fhi, flo], axis=0)              # [12, L]
        fmat_all = np.concatenate([gm2, fhilo], axis=1)         # [12, 128+L]
        in_maps.append({
            "fmat": np.ascontiguousarray(fmat_all),
            "colblk": np.ascontiguousarray(colblk[core].astype(np.float16)),
            "ident": ident,
            "inj": inj_rep,
        })
    meta = dict(L=L, blocks=tuple(blocks), CB=CB, out_chunks=tuple(out_chunks),
                schunks=tuple(schunks))
    return in_maps, meta, out_ranks, slot_order


def _subset_exact(sizes, target):
    """Pick slot ranks whose sizes sum to exactly `target` (DP subset-sum).
    Returns the picked ranks (ascending) or None."""
    n = len(sizes)
    reach = np.zeros((n + 1, target + 1), bool)
    reach[0, 0] = True
    for i in range(n):
        s = int(sizes[i])
        reach[i + 1] = reach[i].copy()
        if s <= target:
            reach[i + 1, s:] |= reach[i, :-s]
    if not reach[n, target]:
        return None
    picked = []
    t = target
    for i in range(n, 0, -1):
        s = int(sizes[i - 1])
        if reach[i - 1, t]:
            continue                               # item i-1 not needed
        picked.append(i - 1)
        t -= s
    assert t == 0
    return sorted(picked)


def _build_program(meta):
    import concourse.bacc as bacc
    import concourse.mybir as mybir
    from concourse.tile import TileContext
    from concourse.mybir import AluOpType

    L = meta["L"]
    blocks = meta["blocks"]
    CB = meta["CB"]
    out_chunks = meta["out_chunks"]

    f32 = mybir.dt.float32
    f16 = mybir.dt.float16
    fm_cols = 128 + L
    nc = bacc.Bacc("TRN2", target_bir_lowering=False)
    f_d = nc.dram_tensor("fmat", [12, fm_cols], f16, kind="ExternalInput")
    cb_d = nc.dram_tensor("colblk", [128, CB], f16, kind="ExternalInput")
    id_d = nc.dram_tensor("ident", [128, 128], f16, kind="ExternalInput")
    inj_d = nc.dram_tensor("inj", [128, L], f16, kind="ExternalInput")
    out_d = nc.dram_tensor("out", [128, 3 * NSLOTS], f32, kind="ExternalOutput")

    # compute chunks for Q/exp/om (PSUM-bank sized)
    qchunks = []
    c0 = 0
    while c0 < L:
        qchunks.append((c0, min(c0 + 512, L)))
        c0 += 512
    # scan chunks (from host packing), all on DVE (GpSimd lacks the scan
    # opcode on TRN2).  No slot straddles a chunk boundary, so every chunk
    # starts fresh with initial=0 -- no chaining.  Execution order is
    # pinned by the om chunks being serial on ACT.
    schunks = meta["schunks"]

    # a block's transpose is ready after the scan chunk covering its end
    blocks_by_schunk = [[] for _ in schunks]
    for blk in blocks:
        bend = (blk[0] + 1) * 128
        for si, (_c0, c1_) in enumerate(schunks):
            if bend <= c1_:
                blocks_by_schunk[si].append(blk)
                break

    with TileContext(nc) as tc:
        with (
            tc.tile_pool(name="const", bufs=1) as cpool,
            tc.tile_pool(name="wts", bufs=2) as wpool,
            tc.tile_pool(name="psum", bufs=2, space="PSUM") as ppool,
            tc.tile_pool(name="trps", bufs=2, space="PSUM") as tpool,
            tc.tile_pool(name="colps", bufs=1, space="PSUM") as opool,
        ):
            fm = cpool.tile([12, fm_cols], f16)
            inj = cpool.tile([128, L], f16)
            cb = cpool.tile([128, CB], f16)
            ident = cpool.tile([128, 128], f16)
            # Everything HWDGE rides the sync queue in consumer order: fm's
            # 12 descriptors enter the DMA rings first (the first matmul
            # gates on them), then inj in two pieces so scan0's gate fires
            # as soon as its half lands, then cb.  Any DMA on the scalar
            # queue floods the shared rings before fm (its gen starts
            # immediately at program entry), so the scalar queue carries no
            # input DMAs at all.
            s_fm = 128 + min(512, L)
            s_inj = min(512, L)
            nc.sync.dma_start(fm[:, :s_fm], f_d[:, :s_fm])
            nc.sync.dma_start(fm[:, s_fm:], f_d[:, s_fm:])
            nc.sync.dma_start(inj[:, :s_inj], inj_d[:, :s_inj])
            if s_inj < L:
                nc.sync.dma_start(inj[:, s_inj:], inj_d[:, s_inj:])
            nc.gpsimd.dma_start(ident[:, :], id_d[:, :])
            nc.sync.dma_start(cb[:, :], cb_d[:, :])

            gm2 = fm[:, 0:128]
            alphat = cpool.tile([128, L], f16)
            omap = cpool.tile([128, L], f16)
            Tt = cpool.tile([128, L], f16)
            colb = cpool.tile([128, 3 * NSLOTS], f32)
            # one PSUM color accumulator per out-chunk: a chunk's clamp
            # (DVE read) then never hazards against a later chunk's color
            # matmuls (PE writes) -- the Tile framework tracks whole tiles
            colps = []
            for si, (oA, oB) in enumerate(out_chunks):
                w3 = max(3 * (oB - oA), 3)
                cp = opool.tile([128, w3], f32, tag=f"colps{si}",
                                name=f"colps{si}")
                nc.vector.memset(cp[:, :], 0.0)
                colps.append(cp)

            # Q matmul + exp per PSUM-bank chunk
            for qi, (c0, c1) in enumerate(qchunks):
                n = c1 - c0
                ps = ppool.tile([128, 512], f32, tag="ps", name="ps")
                nc.tensor.matmul(ps[:, :n], gm2, fm[:, 128 + c0:128 + c1],
                                 start=True, stop=True)
                nc.scalar.activation(alphat[:, c0:c1], ps[:, :n],
                                     mybir.ActivationFunctionType.Exp)

            # om = 1 - alpha per scan chunk (alpha <= 0.99 by host pre-clamp
            # of opacity; Q <= 0 so exp <= 1).  Chunk 0 on DVE (feeds its
            # scan immediately), later chunks on ACT so DVE runs scans
            # back-to-back; the serial ACT order also pins scan order.
            for si, (c0, c1) in enumerate(schunks):
                if si == 0:
                    nc.vector.tensor_scalar(omap[:, c0:c1], alphat[:, c0:c1],
                                            -1.0, 1.0, AluOpType.mult,
                                            AluOpType.add)
                else:
                    nc.scalar.activation(omap[:, c0:c1], alphat[:, c0:c1],
                                         mybir.ActivationFunctionType.Copy,
                                         bias=1.0, scale=-1.0)

            for si, (c0, c1) in enumerate(schunks):
                nc.vector.tensor_tensor_scan(Tt[:, c0:c1], omap[:, c0:c1],
                                             inj[:, c0:c1], 0.0,
                                             AluOpType.mult, AluOpType.max)

                # blocks finalized by this scan: PE transpose -> PSUM,
                # drain -> SBUF, small matmuls accumulate colors, then
                # flush the finalized slots (clamp + store)
                blks = blocks_by_schunk[si]
                last_chunk = si == len(schunks) - 1
                oA, oB = out_chunks[si]
                if blks:
                    nb = len(blks)
                    trp = tpool.tile([128, 512], f16, tag="trp", name="trp")
                    wT = wpool.tile([128, 512], f16, tag="wT", name="wT")
                    for t, (bb, j0, j1, cbo) in enumerate(blks):
                        lo = bb * 128
                        nc.tensor.transpose(trp[:, 128 * t:128 * t + 128],
                                            Tt[:, lo:lo + 128], ident[:, :])
                    span = 128 * nb
                    if si >= 2:
                        # DVE is idle once the scans finish; draining the
                        # short late chunks there skips the busy ACT queue
                        nc.vector.tensor_copy(wT[:, :span], trp[:, :span])
                    else:
                        nc.scalar.copy(wT[:, :span], trp[:, :span])
                    for t, (bb, j0, j1, cbo) in enumerate(blks):
                        assert oA <= j0 and j1 < oB, "block escapes its chunk"
                        k3 = 3 * (j1 - j0 + 1)
                        lo3 = 3 * (j0 - oA)
                        nc.tensor.matmul(colps[si][:, lo3: lo3 + k3],
                                         wT[:, 128 * t:128 * t + 128],
                                         cb[:, cbo: cbo + k3],
                                         start=False, stop=False,
                                         skip_group_check=True)
                # slots fully covered by this chunk are final once its
                # color matmuls are issued; the last chunk's flush rides the
                # scalar queue so its descriptor gen runs in parallel with
                # the sync queue's earlier output DMAs
                if oB > oA:
                    nc.vector.tensor_scalar(colb[:, 3 * oA:3 * oB],
                                            colps[si][:, :3 * (oB - oA)],
                                            0.0, 1.0,
                                            AluOpType.max, AluOpType.min)
                    dq = nc.scalar if last_chunk else nc.sync
                    dq.dma_start(out_d[:, 3 * oA:3 * oB],
                                 colb[:, 3 * oA:3 * oB])
    nc.finalize()
    return nc


def _assemble(results, out_ranks, slot_order):
    out = np.zeros((3, H, W), np.float32)
    dr, dc = np.divmod(np.arange(128), TC)
    for core in range(NCORES):
        o = np.clip(results[core]["out"], 0.0, 1.0)   # [128, 192]
        for oi, r in enumerate(out_ranks):
            ti = int(slot_order[core, r])          # global tile id
            y0 = (ti // NTX) * TR
            x0 = (ti % NTX) * TC
            for ch in range(3):
                out[ch, y0 + dr, x0 + dc] = o[:, 3 * oi + ch]
    return out


def _run(inputs, trace=False, trace_cores=None):
    in_maps, meta, out_ranks, slot_order = _host_prep(
        inputs["positions"], inputs["scales"], inputs["rotations"],
        inputs["colors"], inputs["opacities"], inputs["view_matrix"])

    key = (meta["L"], meta["blocks"], meta["out_chunks"], meta["schunks"])
    if key not in _compile_cache:
        _compile_cache[key] = _build_program(meta)
    nc = _compile_cache[key]

    from concourse.bass_utils import run_bass_kernel_spmd
    kw = {}
    if trace:
        kw = dict(trace=True,
                  trace_cores=trace_cores or list(range(NCORES)))
    res = run_bass_kernel_spmd(nc, in_maps, core_ids=list(range(NCORES)), **kw)
    return _assemble(res.results, out_ranks, slot_order), res


def kernel(**inputs) -> np.ndarray:
    out, _ = _run(inputs, trace=False)
    return out
